# revision 32
# baseline (speedup 1.0000x reference)
"""Trainium2 Bass kernel for nn_BrickVectorEdgeModel (GNN edge MLP).

Computes, for each batch b and node pair (i, j):
    f   = relu(W_b @ relu(W_a @ bv + b_a + W_xy @ xy + b_xy) + b_b)   per node
    e1  = relu(W1 @ f[j] + W2 @ f[i] + b_ca)                          per edge
    e2  = relu(W_cb @ e1 + b_cb)
    e3  = relu(W_cc @ e2 + b_cc)
    out = W_out @ e3 + b_out                                          (2 channels)

Sharding: the (B=4, N=192) x N edge grid has 768 i-rows; each of the 8
cores takes 96 consecutive rows, which always fall inside a single batch
b = core//2.  Host permutes that batch's 192 nodes so the core's own 96
i-rows come first; every core then runs the identical program (SPMD) on
its own node set.  Matmuls run in fp16 (fp32 PSUM accumulate).

Perf structure (vs the naive version):
  - The out layer (512 -> 2) runs as 4 concurrent column-tiled M=2
    matmuls (tile_position=(0,32j)) + one K=98 ones-matmul reduction,
    instead of 4 full-rate M=128 matmuls on a zero-padded weight.
  - PSUM->SBUF drains go to the Scalar engine; e1 construction goes to
    Vector/GpSimd; the out reduction is software-pipelined one chunk
    behind so the PE never waits on the drain copy.
  - Weight DMA is split fine-grained in consumption order; the node
    phase runs k-outer so f1 starts as soon as the first k-tiles land.
  - ~18 dummy warm-up matmuls run during the initial DMA wait to lift
    the PE HAM clock-gate (1.2 -> 2.4 GHz) before the real work.
"""

import numpy as np

import concourse.bass as bass
import concourse.mybir as mybir
import concourse.tile as tile
from concourse import bacc
from concourse.bass_utils import run_bass_kernel_spmd

P = 128
H = 512          # hidden width
D = 512          # brick vector dim
B = 4
N = 192          # nodes per batch
NCORES = 8
RLOC = 96        # edge-grid rows per core
EDGES = RLOC * N             # flat edge columns per core (18432)
CHUNK = 512
NCHUNK = EDGES // CHUNK      # 36
NWARM = 66       # PE warm-up matmuls

# weight blob layout (fp16): name -> (offset_cols, size_cols), [128 x WCOLS]
# w12 holds w1/w2 interleaved per k-tile: [w1k0 w2k0 w1k1 w2k1 ...] so the
# u/v phase (k-outer) can start as soon as the first k-pair lands.
_layout = [
    ("wcat", 5 * H),   # [d_tile(4)+xy_pad(1), 512] stationary tiles for layer a
    ("wb", 4 * H),
    ("w12", 8 * H),
    ("wcb", 4 * H),
    ("wcc", 4 * H),
    ("wout", 4 * 2),   # W_out^T k-tiled: [128, 4, 2]
]
OFF = {}
_c = 0
for _n, _s in _layout:
    OFF[_n] = (_c, _s)
    _c += _s
WCOLS = _c

# DMA stages: (name, start_col, end_col) in consumption order.  Exactly 6
# weight DMAs (+ nodes + biases = 8 total) — the sync queue has 8 DMA
# completion semaphores; a 9th DMA recycles one and serializes on it.
_CUTS = [
    ("wcat", 0, 5 * H),
    ("wb", OFF["wb"][0], OFF["wb"][0] + 4 * H),
    ("w12ab", OFF["w12"][0], OFF["w12"][0] + 4 * H),
    ("w12cd", OFF["w12"][0] + 4 * H, OFF["w12"][0] + 8 * H),
    ("wcb", OFF["wcb"][0], OFF["wcb"][0] + 4 * H),
    ("wcctail", OFF["wcc"][0], WCOLS),              # wcc + wout
]

# bias blob layout (fp32): [128 x BCOLS]
_blayout = [("b1", 4), ("bb", 4), ("bca", 4), ("bcb", 4), ("bcc", 4), ("bout", 1)]
BOFF = {}
_c = 0
for _n, _s in _blayout:
    BOFF[_n] = (_c, _s)
    _c += _s
BCOLS = _c


def _to_tiles(w):
    """[K, M] (K = 4*128 contraction) -> [128, 4, M] stationary layout."""
    K, M = w.shape
    return w.reshape(K // P, P, M).transpose(1, 0, 2)


def _pack_weights(W_xy, b_xy, W_a, b_a, W_b, b_b, W_ca, b_ca, W_cb, b_cb,
                  W_cc, b_cc, W_out, b_out):
    blob = np.zeros((P, WCOLS), np.float16)

    def put(name, arr3):  # arr3: [128, n_k, M]
        off, sz = OFF[name]
        blob[:, off:off + sz] = arr3.reshape(P, -1).astype(np.float16)

    wcat = np.zeros((P, 5, H), np.float32)
    wcat[:, :4, :] = _to_tiles(W_a.T.astype(np.float32))      # [512d, 512h]
    wcat[0:2, 4, :] = W_xy.T.astype(np.float32)               # [2, 512]
    put("wcat", wcat)
    put("wb", _to_tiles(W_b.T.astype(np.float32)))
    W1, W2 = W_ca[:, :H], W_ca[:, H:]
    w1t = _to_tiles(W1.T.astype(np.float32))          # [128, 4, 512]
    w2t = _to_tiles(W2.T.astype(np.float32))
    w12 = np.stack([w1t, w2t], axis=2)                # [128, 4, 2, 512]
    put("w12", w12)
    put("wcb", _to_tiles(W_cb.T.astype(np.float32)))
    put("wcc", _to_tiles(W_cc.T.astype(np.float32)))
    put("wout", _to_tiles(W_out.T.astype(np.float32)))        # [128, 4, 2]

    bblob = np.zeros((P, BCOLS), np.float32)

    def putb(name, vec):  # [512] -> [128, 4]
        off, sz = BOFF[name]
        bblob[:, off:off + sz] = vec.astype(np.float32).reshape(4, P).T

    putb("b1", np.asarray(b_a) + np.asarray(b_xy))
    putb("bb", b_b)
    putb("bca", b_ca)
    putb("bcb", b_cb)
    putb("bcc", b_cc)
    off, _ = BOFF["bout"]
    bblob[0:2, off] = np.asarray(b_out, np.float32)
    return blob, bblob


def _pack_nodes(bv_b, xy_b, perm):
    """Per-core node blob [128, 5, N] fp16: k-tiles 0-3 = bv^T, 4 = xy^T."""
    nb = np.zeros((P, 5, N), np.float16)
    bvT = bv_b[perm].T.astype(np.float32)          # [512, 192]
    nb[:, 0:4, :] = bvT.reshape(4, P, N).transpose(1, 0, 2).astype(np.float16)
    nb[0:2, 4, :] = xy_b[perm].T.astype(np.float16)
    return nb


def _build():
    f32 = mybir.dt.float32
    Relu = mybir.ActivationFunctionType.Relu
    add = mybir.AluOpType.add
    amax = mybir.AluOpType.max

    f16 = mybir.dt.float16
    nc = bacc.Bacc(None, target_bir_lowering=False)
    wblob = nc.declare_dram_parameter("wblob", [P, WCOLS], f16, isOutput=False)
    bblob = nc.declare_dram_parameter("bblob", [P, BCOLS], f32, isOutput=False)
    nodes = nc.declare_dram_parameter("nodes", [P, 5, N], f16, isOutput=False)
    y = nc.declare_dram_parameter("y", [2, EDGES], f32, isOutput=True)

    with tile.TileContext(nc) as tc:
        with tc.tile_pool(name="wf", bufs=1) as wf, \
             tc.tile_pool(name="stp", bufs=1) as stp, \
             tc.tile_pool(name="wr", bufs=1) as wr, \
             tc.tile_pool(name="ep", bufs=3) as ep, \
             tc.tile_pool(name="sbp", bufs=1) as sbp, \
             tc.tile_pool(name="outp", bufs=3) as outp, \
             tc.tile_pool(name="psA", bufs=3, space="PSUM") as psA, \
             tc.tile_pool(name="psB", bufs=3, space="PSUM") as psB, \
             tc.tile_pool(name="psO", bufs=2, space="PSUM") as psO:

            bias_t = wf.tile([P, BCOLS], f32, tag="bias")

            def bias(name, m):
                off, _ = BOFF[name]
                return bias_t[:, off + m:off + m + 1]

            # ---- weight/input DMAs, in consumption order ----
            nd_r = wf.tile([P, 5, N], f16, tag="nodes")
            nc.sync.dma_start(nd_r[:], nodes[:])
            stg = {}
            for nm, c0, c1 in _CUTS[:1]:
                stg[nm] = stp.tile([P, c1 - c0], f16, tag=nm, name=f"st_{nm}")
                nc.sync.dma_start(stg[nm][:], wblob[:, c0:c1])
            nc.sync.dma_start(bias_t[:], bblob[:])
            for nm, c0, c1 in _CUTS[1:]:
                stg[nm] = stp.tile([P, c1 - c0], f16, tag=nm, name=f"st_{nm}")
                # edge-layer weights go on the Scalar HWDGE queue so they
                # stream concurrently with the node-phase weights and are
                # resident well before the first e2/e3 matmul needs them.
                eng = nc.scalar if nm in ("wcb", "wcctail") else nc.sync
                eng.dma_start(stg[nm][:], wblob[:, c0:c1])

            def wslice(name, nk, m):
                off, sz = OFF[name]
                assert sz == nk * m
                for nm, c0, c1 in _CUTS:
                    if c0 <= off and off + sz <= c1:
                        return stg[nm][:, off - c0:off - c0 + sz].rearrange(
                            "p (a b) -> p a b", b=m)
                raise AssertionError(name)

            wcat_t = stg["wcat"][:].rearrange("p (a b) -> p a b", b=H)
            wb = wslice("wb", 4, H)
            wcb = wslice("wcb", 4, H)
            wcc = wslice("wcc", 4, H)
            wout = wslice("wout", 4, 2)

            def wcat(k):
                return wcat_t[:, k, :]

            def w1k(k):
                st = stg["w12ab"] if k < 2 else stg["w12cd"]
                return st[:, (k % 2) * 2 * H:(k % 2) * 2 * H + H]

            def w2k(k):
                st = stg["w12ab"] if k < 2 else stg["w12cd"]
                return st[:, (k % 2) * 2 * H + H:(k % 2) * 2 * H + 2 * H]

            # ---- PE warm-up (lifts HAM clock gate during the DMA wait) ----
            warm = wf.tile([P, P], f16, tag="warm")
            nc.gpsimd.memset(warm[:], 0.001)
            poA = psO.tile([P, CHUNK], f32, tag="po")
            poB = psO.tile([P, CHUNK], f32, tag="po")
            for _ in range(NWARM):
                nc.tensor.matmul(poA[0:2, 0:P], warm[:, 0:2], warm[:],
                                 start=True, stop=True)

            # ---- node phase: f1, f2 (k-outer), then u / vpb ----
            def kouter(dst, nkt, lhs_of_k, rhs_of_k, bname, cast_m):
                pts = [psA.tile([P, CHUNK], f32, tag="psA", name=f"pf{m}")
                       if m < 3 else
                       psB.tile([P, CHUNK], f32, tag="psB", name=f"pf{m}")
                       for m in range(4)]
                for k in range(nkt):
                    for m in range(4):
                        nc.tensor.matmul(pts[m][:, :N], lhs_of_k(k, m),
                                         rhs_of_k(k), start=(k == 0),
                                         stop=(k == nkt - 1))
                for m in range(4):
                    cast_m(m, pts[m])

            f1 = wr.tile([P, 4, N], f16, tag="f1")
            kouter(f1, 5,
                   lambda k, m: wcat(k)[:, m * P:(m + 1) * P],
                   lambda k: nd_r[:, k, :],
                   "b1",
                   lambda m, pt: nc.scalar.activation(
                       f1[:, m, :], pt[:, :N], Relu, bias=bias("b1", m),
                       scale=1.0))

            f2 = wr.tile([P, 4, N], f16, tag="f2")
            kouter(f2, 4,
                   lambda k, m: wb[:, k, m * P:(m + 1) * P],
                   lambda k: f1[:, k, :],
                   "bb",
                   lambda m, pt: nc.scalar.activation(
                       f2[:, m, :], pt[:, :N], Relu, bias=bias("bb", m),
                       scale=1.0))

            # u / vpb: k-outer over all 8 m-tiles so the first w12 k-pair DMA
            # unblocks the whole phase; uses all 8 PSUM banks.
            u = wr.tile([P, 4, N], f32, tag="u")
            vpb = wr.tile([P, 4, N], f32, tag="vpb")
            ups = [psA.tile([P, CHUNK], f32, tag="psA", name=f"pu{m}")
                   if m < 3 else
                   psB.tile([P, CHUNK], f32, tag="psB", name=f"pu{m}")
                   for m in range(4)]
            vps = [psB.tile([P, CHUNK], f32, tag="psB", name=f"pv{m}")
                   for m in range(2)] + [poA, poB]
            for k in range(4):
                for m in range(4):
                    nc.tensor.matmul(ups[m][:, :N], w1k(k)[:, m * P:(m + 1) * P],
                                     f2[:, k, :], start=(k == 0), stop=(k == 3))
                for m in range(4):
                    nc.tensor.matmul(vps[m][:, :N], w2k(k)[:, m * P:(m + 1) * P],
                                     f2[:, k, :], start=(k == 0), stop=(k == 3))
            # drain u/vpb per m-tile and immediately build chunk 0's e1 for
            # that k-tile, so the first e2 matmul isn't gated on the whole
            # serial Vector drain chain.
            e1_c0 = ep.tile([P, 4, CHUNK], f16, tag="e1")

            def e1_segs(e1t, kt, f0, cw):
                r_lo, r_hi = f0 // N, (f0 + cw - 1) // N
                for rl in range(r_lo, r_hi + 1):
                    cs = max(f0, rl * N)
                    ce = min(f0 + cw, (rl + 1) * N)
                    nc.vector.tensor_scalar(
                        e1t[:, kt, cs - f0:ce - f0],
                        u[:, kt, cs - rl * N:ce - rl * N],
                        vpb[:, kt, rl:rl + 1], 0.0, add, amax)

            for m in range(4):
                if m < 2:
                    nc.scalar.copy(u[:, m, :], ups[m][:, :N])
                else:
                    nc.vector.tensor_copy(u[:, m, :], ups[m][:, :N])
                nc.vector.tensor_scalar_add(vpb[:, m, :], vps[m][:, :N],
                                            bias("bca", m))
                e1_segs(e1_c0, m, 0, CHUNK)

            # ---- edge phase ----
            chunk_list = [(cc * CHUNK, CHUNK) for cc in range(NCHUNK - 1)]
            chunk_list += [((NCHUNK - 1) * CHUNK, CHUNK // 2),
                           ((NCHUNK - 1) * CHUNK + CHUNK // 2, CHUNK // 2)]

            x01 = sbp.tile([2, CHUNK], f32, tag="x01")
            colq = []  # [(cc, e3, f0, cw)] chunks awaiting the col-MM group
            redq = []  # [(cc, f0, cw)] chunks awaiting the 2-way add + dma

            def emit_colgrp():
                # out matmuls for the previous chunk as two concurrent
                # column-position groups, each PSUM-accumulating a k-pair;
                # its e3 drains finished during this chunk's e2 m0 block.
                cc_, e3_, f0_, cw_ = colq.pop(0)
                po_ = poA if cc_ % 2 == 0 else poB
                for pos, ja in ((0, 0), (32, 2)):
                    for j in (ja, ja + 1):
                        nc.tensor.matmul(po_[pos:pos + 2, :cw_], wout[:, j, :],
                                         e3_[:, j, :cw_], start=(j == ja),
                                         stop=(j == ja + 1),
                                         tile_position=(0, pos))
                redq.append((cc_, f0_, cw_))

            def emit_reduce():
                # fold the two column-position partials; b_out is added on
                # the host.  (DVE may read at most one PSUM operand, so the
                # second partial goes through SBUF via the Scalar engine.)
                cc_, f0_, cw_ = redq.pop(0)
                po_ = poA if cc_ % 2 == 0 else poB
                nc.scalar.copy(x01[:, :cw_], po_[32:34, :cw_])
                ob = outp.tile([2, CHUNK], f32, tag="ob")
                nc.vector.tensor_tensor(ob[:, :cw_], po_[0:2, :cw_],
                                        x01[:, :cw_], add)
                nc.sync.dma_start(y[:, f0_:f0_ + cw_], ob[:, :cw_])

            for cc, (f0, cw) in enumerate(chunk_list):
                if cc == 0:
                    e1 = e1_c0
                else:
                    e1 = ep.tile([P, 4, CHUNK], f16, tag="e1")
                    for kt in range(4):
                        e1_segs(e1, kt, f0, cw)

                e2 = ep.tile([P, 4, CHUNK], f16, tag="e2")
                for m in range(4):
                    pt = psA.tile([P, CHUNK], f32, tag="psA")
                    for k in range(4):
                        nc.tensor.matmul(pt[:, :cw], wcb[:, k, m * P:(m + 1) * P],
                                         e1[:, k, :cw], start=(k == 0), stop=(k == 3))
                    nc.scalar.activation(e2[:, m, :cw], pt[:, :cw], Relu,
                                         bias=bias("bcb", m), scale=1.0)
                    if m == 0 and colq:
                        emit_colgrp()

                e3 = ep.tile([P, 4, CHUNK], f16, tag="e3")
                for m in range(4):
                    pt = psB.tile([P, CHUNK], f32, tag="psB")
                    for k in range(4):
                        nc.tensor.matmul(pt[:, :cw], wcc[:, k, m * P:(m + 1) * P],
                                         e2[:, k, :cw], start=(k == 0), stop=(k == 3))
                    nc.scalar.activation(e3[:, m, :cw], pt[:, :cw], Relu,
                                         bias=bias("bcc", m), scale=1.0)

                if len(redq) >= 2 or (redq and redq[0][0] <= cc - 2):
                    emit_reduce()
                colq.append((cc, e3, f0, cw))

            while colq:
                emit_colgrp()
            while redq:
                emit_reduce()

    nc.compile()
    return nc


_cache = {}


def _get_nc():
    if "nc" not in _cache:
        _cache["nc"] = _build()
    return _cache["nc"]


def kernel(brick_vectors, xy, W_xy, b_xy, W_a, b_a, W_b, b_b,
           W_ca, b_ca, W_cb, b_cb, W_cc, b_cc, W_out, b_out):
    # force plain numpy up front (inputs may arrive as jax arrays)
    brick_vectors = np.asarray(brick_vectors, np.float32)
    xy = np.asarray(xy, np.float32)
    W_xy, b_xy, W_a, b_a = map(np.asarray, (W_xy, b_xy, W_a, b_a))
    W_b, b_b, W_ca, b_ca = map(np.asarray, (W_b, b_b, W_ca, b_ca))
    W_cb, b_cb, W_cc, b_cc = map(np.asarray, (W_cb, b_cb, W_cc, b_cc))
    W_out, b_out = np.asarray(W_out), np.asarray(b_out)
    blob, bblob = _pack_weights(W_xy, b_xy, W_a, b_a, W_b, b_b, W_ca, b_ca,
                                W_cb, b_cb, W_cc, b_cc, W_out, b_out)

    perms = []
    in_maps = []
    for c in range(NCORES):
        b, half = c // 2, c % 2
        perm = np.concatenate([np.arange(96) + 96 * half,
                               np.arange(96) + 96 * (1 - half)])
        perms.append((b, perm))
        in_maps.append({
            "wblob": blob,
            "bblob": bblob,
            "nodes": _pack_nodes(brick_vectors[b], xy[b], perm),
        })

    nc = _get_nc()
    res = run_bass_kernel_spmd(nc, in_maps, list(range(NCORES)))

    bout = np.asarray(b_out, np.float32).reshape(1, 1, 2)
    out = np.empty((B, N, N, 2), np.float32)
    for c in range(NCORES):
        b, perm = perms[c]
        yc = res.results[c]["y"].reshape(2, RLOC, N)       # [2, rl, jj]
        out[b][np.ix_(perm[:RLOC], perm)] = yc.transpose(1, 2, 0) + bout
    return out


# revision 45
# speedup vs baseline: 1.0029x; 1.0029x over previous
"""Trainium2 Bass kernel for nn_BrickVectorEdgeModel (GNN edge MLP).

Computes, for each batch b and node pair (i, j):
    f   = relu(W_b @ relu(W_a @ bv + b_a + W_xy @ xy + b_xy) + b_b)   per node
    e1  = relu(W1 @ f[j] + W2 @ f[i] + b_ca)                          per edge
    e2  = relu(W_cb @ e1 + b_cb)
    e3  = relu(W_cc @ e2 + b_cc)
    out = W_out @ e3 + b_out                                          (2 channels)

Sharding: the (B=4, N=192) x N edge grid has 768 i-rows; each of the 8
cores takes 96 consecutive rows, which always fall inside a single batch
b = core//2.  Host permutes that batch's 192 nodes so the core's own 96
i-rows come first; every core then runs the identical program (SPMD) on
its own node set.  Matmuls run in fp16 (fp32 PSUM accumulate).

Perf structure (vs the naive version):
  - The out layer (512 -> 2) runs as two concurrent column-position
    matmul groups (tile_position=(0,0)/(0,32), M=2, each PSUM-
    accumulating a k-pair), folded by one Scalar copy + one Vector add;
    b_out is applied on the host.  This replaces 4 full-rate M=128
    matmuls on a zero-padded weight and is software-pipelined a chunk
    behind the e3 drains so the PE never waits.
  - PSUM->SBUF drains go to the Scalar engine; e1 construction runs on
    Vector (2x_2P fast path; GpSimd is ~10x too slow for this op).
  - Weight DMA: 8 transfers on the sync queue (its semaphore-pool
    depth) in consumption order; wcb/wcc ride the Scalar HWDGE queue
    concurrently; the node phase runs k-outer over all 8 PSUM banks.
  - Dummy warm-up matmuls during the initial DMA wait lift the PE HAM
    clock-gate (1.2 -> 2.4 GHz) before the real work.
"""

import numpy as np

import concourse.bass as bass
import concourse.mybir as mybir
import concourse.tile as tile
from concourse import bacc
from concourse.bass_utils import run_bass_kernel_spmd

P = 128
H = 512          # hidden width
D = 512          # brick vector dim
B = 4
N = 192          # nodes per batch
NCORES = 8
RLOC = 96        # edge-grid rows per core
EDGES = RLOC * N             # flat edge columns per core (18432)
CHUNK = 512
NCHUNK = EDGES // CHUNK      # 36
NWARM = 60       # PE warm-up matmuls

# weight blob layout (fp16): name -> (offset_cols, size_cols), [128 x WCOLS]
# w12 holds w1/w2 interleaved per k-tile: [w1k0 w2k0 w1k1 w2k1 ...] so the
# u/v phase (k-outer) can start as soon as the first k-pair lands.
_layout = [
    ("wcat", 4 * H),   # [d_tile, 512] stationary tiles for layer a (bv part)
    ("wxy", H),        # W_xy^T on partitions 0-1 only (K=2 contraction)
    ("wb", 4 * H),
    ("w12", 8 * H),
    ("wcb", 4 * H),
    ("wcc", 4 * H),
    ("wout", 4 * 2),   # W_out^T k-tiled: [128, 4, 2]
]
OFF = {}
_c = 0
for _n, _s in _layout:
    OFF[_n] = (_c, _s)
    _c += _s
WCOLS = _c

# DMA stages: (name, start_col, end_col) in consumption order.  Exactly 6
# weight DMAs (+ nodes + biases = 8 total) — the sync queue has 8 DMA
# completion semaphores; a 9th DMA recycles one and serializes on it.
_CUTS = [
    ("wcat", 0, 4 * H),
    ("wb", OFF["wb"][0], OFF["wb"][0] + 4 * H),
    ("w12ab", OFF["w12"][0], OFF["w12"][0] + 4 * H),
    ("w12cd", OFF["w12"][0] + 4 * H, OFF["w12"][0] + 8 * H),
    ("wcb", OFF["wcb"][0], OFF["wcb"][0] + 4 * H),
    ("wcctail", OFF["wcc"][0], WCOLS),              # wcc + wout
]

# bias blob layout (fp32): [128 x BCOLS]
_blayout = [("b1", 4), ("bb", 4), ("bca", 4), ("bcb", 4), ("bcc", 4), ("bout", 1)]
BOFF = {}
_c = 0
for _n, _s in _blayout:
    BOFF[_n] = (_c, _s)
    _c += _s
BCOLS = _c


def _to_tiles(w):
    """[K, M] (K = 4*128 contraction) -> [128, 4, M] stationary layout."""
    K, M = w.shape
    return w.reshape(K // P, P, M).transpose(1, 0, 2)


def _pack_weights(W_xy, b_xy, W_a, b_a, W_b, b_b, W_ca, b_ca, W_cb, b_cb,
                  W_cc, b_cc, W_out, b_out):
    blob = np.zeros((P, WCOLS), np.float16)

    def put(name, arr3):  # arr3: [128, n_k, M]
        off, sz = OFF[name]
        blob[:, off:off + sz] = arr3.reshape(P, -1).astype(np.float16)

    put("wcat", _to_tiles(W_a.T.astype(np.float32)))          # [512d, 512h]
    off, _ = OFF["wxy"]
    blob[0:2, off:off + H] = W_xy.T.astype(np.float16)        # [2, 512]
    put("wb", _to_tiles(W_b.T.astype(np.float32)))
    W1, W2 = W_ca[:, :H], W_ca[:, H:]
    w1t = _to_tiles(W1.T.astype(np.float32))          # [128, 4, 512]
    w2t = _to_tiles(W2.T.astype(np.float32))
    w12 = np.stack([w1t, w2t], axis=2)                # [128, 4, 2, 512]
    put("w12", w12)
    put("wcb", _to_tiles(W_cb.T.astype(np.float32)))
    put("wcc", _to_tiles(W_cc.T.astype(np.float32)))
    put("wout", _to_tiles(W_out.T.astype(np.float32)))        # [128, 4, 2]

    bblob = np.zeros((P, BCOLS), np.float32)

    def putb(name, vec):  # [512] -> [128, 4]
        off, sz = BOFF[name]
        bblob[:, off:off + sz] = vec.astype(np.float32).reshape(4, P).T

    putb("b1", np.asarray(b_a) + np.asarray(b_xy))
    putb("bb", b_b)
    putb("bca", b_ca)
    putb("bcb", b_cb)
    putb("bcc", b_cc)
    off, _ = BOFF["bout"]
    bblob[0:2, off] = np.asarray(b_out, np.float32)
    return blob, bblob


def _pack_nodes(bv_b, xy_b, perm):
    """Per-core node blob [128, 5, N] fp16: k-tiles 0-3 = bv^T, 4 = xy^T."""
    nb = np.zeros((P, 5, N), np.float16)
    bvT = bv_b[perm].T.astype(np.float32)          # [512, 192]
    nb[:, 0:4, :] = bvT.reshape(4, P, N).transpose(1, 0, 2).astype(np.float16)
    nb[0:2, 4, :] = xy_b[perm].T.astype(np.float16)
    return nb


def _build():
    f32 = mybir.dt.float32
    Relu = mybir.ActivationFunctionType.Relu
    add = mybir.AluOpType.add
    amax = mybir.AluOpType.max

    f16 = mybir.dt.float16
    nc = bacc.Bacc(None, target_bir_lowering=False)
    wblob = nc.declare_dram_parameter("wblob", [P, WCOLS], f16, isOutput=False)
    bblob = nc.declare_dram_parameter("bblob", [P, BCOLS], f32, isOutput=False)
    nodes = nc.declare_dram_parameter("nodes", [P, 5, N], f16, isOutput=False)
    y = nc.declare_dram_parameter("y", [2, EDGES], f32, isOutput=True)

    with tile.TileContext(nc) as tc:
        with tc.tile_pool(name="wf", bufs=1) as wf, \
             tc.tile_pool(name="stp", bufs=1) as stp, \
             tc.tile_pool(name="wr", bufs=1) as wr, \
             tc.tile_pool(name="ep", bufs=3) as ep, \
             tc.tile_pool(name="sbp", bufs=1) as sbp, \
             tc.tile_pool(name="outp", bufs=3) as outp, \
             tc.tile_pool(name="psA", bufs=3, space="PSUM") as psA, \
             tc.tile_pool(name="psB", bufs=3, space="PSUM") as psB, \
             tc.tile_pool(name="psO", bufs=2, space="PSUM") as psO:

            bias_t = wf.tile([P, BCOLS], f32, tag="bias")

            def bias(name, m):
                off, _ = BOFF[name]
                return bias_t[:, off + m:off + m + 1]

            # ---- weight/input DMAs, in consumption order ----
            nd_r = wf.tile([P, 5, N], f16, tag="nodes")
            nc.sync.dma_start(nd_r[:], nodes[:])
            stg = {}
            for nm, c0, c1 in _CUTS[:1]:
                stg[nm] = stp.tile([P, c1 - c0], f16, tag=nm, name=f"st_{nm}")
                nc.sync.dma_start(stg[nm][:], wblob[:, c0:c1])
            # W_xy occupies partitions 0-1 only — a 2-row DMA instead of a
            # zero-padded full k-tile keeps 129KB off the critical queue.
            xyw = stp.tile([P, H], f16, tag="wxy", name="st_wxy")
            nc.scalar.dma_start(xyw[0:2, :],
                                wblob[0:2, OFF["wxy"][0]:OFF["wxy"][0] + H])
            nc.scalar.dma_start(bias_t[:], bblob[:])
            for nm, c0, c1 in _CUTS[1:]:
                stg[nm] = stp.tile([P, c1 - c0], f16, tag=nm, name=f"st_{nm}")
                # edge-layer weights go on the Scalar HWDGE queue so they
                # stream concurrently with the node-phase weights and are
                # resident well before the first e2/e3 matmul needs them.
                eng = nc.scalar if nm in ("wcb", "wcctail") else nc.sync
                eng.dma_start(stg[nm][:], wblob[:, c0:c1])

            def wslice(name, nk, m):
                off, sz = OFF[name]
                assert sz == nk * m
                for nm, c0, c1 in _CUTS:
                    if c0 <= off and off + sz <= c1:
                        return stg[nm][:, off - c0:off - c0 + sz].rearrange(
                            "p (a b) -> p a b", b=m)
                raise AssertionError(name)

            wcat_t = stg["wcat"][:].rearrange("p (a b) -> p a b", b=H)
            wb = wslice("wb", 4, H)
            assert OFF["wxy"][1] == H
            wcb = wslice("wcb", 4, H)
            wcc = wslice("wcc", 4, H)
            wout = wslice("wout", 4, 2)

            def wcat(k):
                return wcat_t[:, k, :] if k < 4 else xyw[0:2, :]

            def w1k(k):
                st = stg["w12ab"] if k < 2 else stg["w12cd"]
                return st[:, (k % 2) * 2 * H:(k % 2) * 2 * H + H]

            def w2k(k):
                st = stg["w12ab"] if k < 2 else stg["w12cd"]
                return st[:, (k % 2) * 2 * H + H:(k % 2) * 2 * H + 2 * H]

            # ---- PE warm-up (lifts HAM clock gate during the DMA wait) ----
            warm = wf.tile([P, P], f16, tag="warm")
            nc.gpsimd.memset(warm[:], 0.001)
            poA = psO.tile([P, CHUNK], f32, tag="po")
            poB = psO.tile([P, CHUNK], f32, tag="po")
            for _ in range(NWARM):
                nc.tensor.matmul(poA[0:2, 0:P], warm[:, 0:2], warm[:],
                                 start=True, stop=True)

            # ---- node phase: f1, f2 (k-outer), then u / vpb ----
            def kouter(dst, nkt, lhs_of_k, rhs_of_k, bname, cast_m):
                pts = [psA.tile([P, CHUNK], f32, tag="psA", name=f"pf{m}")
                       if m < 3 else
                       psB.tile([P, CHUNK], f32, tag="psB", name=f"pf{m}")
                       for m in range(4)]
                for k in range(nkt):
                    for m in range(4):
                        nc.tensor.matmul(pts[m][:, :N], lhs_of_k(k, m),
                                         rhs_of_k(k), start=(k == 0),
                                         stop=(k == nkt - 1))
                for m in range(4):
                    cast_m(m, pts[m])

            f1 = wr.tile([P, 4, N], f16, tag="f1")
            kouter(f1, 5,
                   lambda k, m: wcat(k)[:, m * P:(m + 1) * P],
                   lambda k: nd_r[:, k, :] if k < 4 else nd_r[0:2, 4, :],
                   "b1",
                   lambda m, pt: nc.scalar.activation(
                       f1[:, m, :], pt[:, :N], Relu, bias=bias("b1", m),
                       scale=1.0))

            f2 = wr.tile([P, 4, N], f16, tag="f2")
            kouter(f2, 4,
                   lambda k, m: wb[:, k, m * P:(m + 1) * P],
                   lambda k: f1[:, k, :],
                   "bb",
                   lambda m, pt: nc.scalar.activation(
                       f2[:, m, :], pt[:, :N], Relu, bias=bias("bb", m),
                       scale=1.0))

            # u / vpb: k-outer over all 8 m-tiles so the first w12 k-pair DMA
            # unblocks the whole phase; uses all 8 PSUM banks.
            u = wr.tile([P, 4, N], f32, tag="u")
            vpb = wr.tile([P, 4, N], f32, tag="vpb")
            ups = [psA.tile([P, CHUNK], f32, tag="psA", name=f"pu{m}")
                   if m < 3 else
                   psB.tile([P, CHUNK], f32, tag="psB", name=f"pu{m}")
                   for m in range(4)]
            vps = [psB.tile([P, CHUNK], f32, tag="psB", name=f"pv{m}")
                   for m in range(2)] + [poA, poB]
            for k in range(4):
                for m in range(4):
                    nc.tensor.matmul(ups[m][:, :N], w1k(k)[:, m * P:(m + 1) * P],
                                     f2[:, k, :], start=(k == 0), stop=(k == 3))
                for m in range(4):
                    nc.tensor.matmul(vps[m][:, :N], w2k(k)[:, m * P:(m + 1) * P],
                                     f2[:, k, :], start=(k == 0), stop=(k == 3))
            def e1_segs(e1t, kt, f0, cw, eng):
                r_lo, r_hi = f0 // N, (f0 + cw - 1) // N
                for rl in range(r_lo, r_hi + 1):
                    cs = max(f0, rl * N)
                    ce = min(f0 + cw, (rl + 1) * N)
                    if eng is nc.scalar:
                        nc.scalar.activation(
                            e1t[:, kt, cs - f0:ce - f0],
                            u[:, kt, cs - rl * N:ce - rl * N],
                            Relu, bias=vpb[:, kt, rl:rl + 1], scale=1.0)
                    else:
                        eng.tensor_scalar(
                            e1t[:, kt, cs - f0:ce - f0],
                            u[:, kt, cs - rl * N:ce - rl * N],
                            vpb[:, kt, rl:rl + 1], 0.0, add, amax)

            # drain u/vpb and build chunk 0's e1 with the load split across
            # Scalar (kt 0-1) and Vector (kt 2-3) so the first e2 matmul
            # isn't gated on one engine's serial chain.
            e1_c0 = ep.tile([P, 4, CHUNK], f16, tag="e1")
            nc.scalar.copy(u[:, 0, :], ups[0][:, :N])
            nc.vector.tensor_scalar_add(vpb[:, 0, :], vps[0][:, :N],
                                        bias("bca", 0))
            nc.scalar.copy(u[:, 1, :], ups[1][:, :N])
            nc.vector.tensor_scalar_add(vpb[:, 1, :], vps[1][:, :N],
                                        bias("bca", 1))
            e1_segs(e1_c0, 0, 0, CHUNK, nc.scalar)
            nc.vector.tensor_copy(u[:, 2, :], ups[2][:, :N])
            nc.vector.tensor_scalar_add(vpb[:, 2, :], vps[2][:, :N],
                                        bias("bca", 2))
            e1_segs(e1_c0, 1, 0, CHUNK, nc.scalar)
            nc.vector.tensor_copy(u[:, 3, :], ups[3][:, :N])
            nc.vector.tensor_scalar_add(vpb[:, 3, :], vps[3][:, :N],
                                        bias("bca", 3))
            e1_segs(e1_c0, 2, 0, CHUNK, nc.vector)
            e1_segs(e1_c0, 3, 0, CHUNK, nc.vector)

            # ---- edge phase ----
            chunk_list = [(cc * CHUNK, CHUNK) for cc in range(NCHUNK - 1)]
            chunk_list += [((NCHUNK - 1) * CHUNK, CHUNK // 2),
                           ((NCHUNK - 1) * CHUNK + CHUNK // 2, CHUNK // 2)]

            x01 = sbp.tile([2, CHUNK], f32, tag="x01")
            colq = []  # [(cc, e3, f0, cw)] chunks awaiting the col-MM group
            redq = []  # [(cc, f0, cw)] chunks awaiting the 2-way add + dma

            def emit_colgrp():
                # out matmuls for the previous chunk as two concurrent
                # column-position groups, each PSUM-accumulating a k-pair;
                # its e3 drains finished during this chunk's e2 m0 block.
                cc_, e3_, f0_, cw_ = colq.pop(0)
                po_ = poA if cc_ % 2 == 0 else poB
                for pos, ja in ((0, 0), (32, 2)):
                    for j in (ja, ja + 1):
                        nc.tensor.matmul(po_[pos:pos + 2, :cw_], wout[:, j, :],
                                         e3_[:, j, :cw_], start=(j == ja),
                                         stop=(j == ja + 1),
                                         tile_position=(0, pos))
                redq.append((cc_, f0_, cw_))

            def emit_reduce():
                # fold the two column-position partials; b_out is added on
                # the host.  (DVE may read at most one PSUM operand, so the
                # second partial goes through SBUF via the Scalar engine.)
                cc_, f0_, cw_ = redq.pop(0)
                po_ = poA if cc_ % 2 == 0 else poB
                nc.scalar.copy(x01[:, :cw_], po_[32:34, :cw_])
                ob = outp.tile([2, CHUNK], f32, tag="ob")
                nc.vector.tensor_tensor(ob[:, :cw_], po_[0:2, :cw_],
                                        x01[:, :cw_], add)
                nc.sync.dma_start(y[:, f0_:f0_ + cw_], ob[:, :cw_])

            for cc, (f0, cw) in enumerate(chunk_list):
                if cc == 0:
                    e1 = e1_c0
                else:
                    e1 = ep.tile([P, 4, CHUNK], f16, tag="e1")
                    for kt in range(4):
                        e1_segs(e1, kt, f0, cw, nc.vector)

                e2 = ep.tile([P, 4, CHUNK], f16, tag="e2")
                for m in range(4):
                    pt = psA.tile([P, CHUNK], f32, tag="psA")
                    for k in range(4):
                        nc.tensor.matmul(pt[:, :cw], wcb[:, k, m * P:(m + 1) * P],
                                         e1[:, k, :cw], start=(k == 0), stop=(k == 3))
                    nc.scalar.activation(e2[:, m, :cw], pt[:, :cw], Relu,
                                         bias=bias("bcb", m), scale=1.0)
                    if m == 0 and colq:
                        emit_colgrp()

                e3 = ep.tile([P, 4, CHUNK], f16, tag="e3")
                for m in range(4):
                    pt = psB.tile([P, CHUNK], f32, tag="psB")
                    for k in range(4):
                        nc.tensor.matmul(pt[:, :cw], wcc[:, k, m * P:(m + 1) * P],
                                         e2[:, k, :cw], start=(k == 0), stop=(k == 3))
                    nc.scalar.activation(e3[:, m, :cw], pt[:, :cw], Relu,
                                         bias=bias("bcc", m), scale=1.0)

                if len(redq) >= 2 or (redq and redq[0][0] <= cc - 2):
                    emit_reduce()
                colq.append((cc, e3, f0, cw))

            if redq:
                emit_reduce()   # cover the final colgrp's e3-drain wait
            while colq:
                emit_colgrp()
            while redq:
                emit_reduce()

    nc.compile()
    return nc


_cache = {}


def _get_nc():
    if "nc" not in _cache:
        _cache["nc"] = _build()
    return _cache["nc"]


def kernel(brick_vectors, xy, W_xy, b_xy, W_a, b_a, W_b, b_b,
           W_ca, b_ca, W_cb, b_cb, W_cc, b_cc, W_out, b_out):
    # force plain numpy up front (inputs may arrive as jax arrays)
    brick_vectors = np.asarray(brick_vectors, np.float32)
    xy = np.asarray(xy, np.float32)
    W_xy, b_xy, W_a, b_a = map(np.asarray, (W_xy, b_xy, W_a, b_a))
    W_b, b_b, W_ca, b_ca = map(np.asarray, (W_b, b_b, W_ca, b_ca))
    W_cb, b_cb, W_cc, b_cc = map(np.asarray, (W_cb, b_cb, W_cc, b_cc))
    W_out, b_out = np.asarray(W_out), np.asarray(b_out)
    blob, bblob = _pack_weights(W_xy, b_xy, W_a, b_a, W_b, b_b, W_ca, b_ca,
                                W_cb, b_cb, W_cc, b_cc, W_out, b_out)

    perms = []
    in_maps = []
    for c in range(NCORES):
        b, half = c // 2, c % 2
        perm = np.concatenate([np.arange(96) + 96 * half,
                               np.arange(96) + 96 * (1 - half)])
        perms.append((b, perm))
        in_maps.append({
            "wblob": blob,
            "bblob": bblob,
            "nodes": _pack_nodes(brick_vectors[b], xy[b], perm),
        })

    nc = _get_nc()
    res = run_bass_kernel_spmd(nc, in_maps, list(range(NCORES)))

    bout = np.asarray(b_out, np.float32).reshape(1, 1, 2)
    out = np.empty((B, N, N, 2), np.float32)
    for c in range(NCORES):
        b, perm = perms[c]
        yc = res.results[c]["y"].reshape(2, RLOC, N)       # [2, rl, jj]
        out[b][np.ix_(perm[:RLOC], perm)] = yc.transpose(1, 2, 0) + bout
    return out
